# revision 68
# baseline (speedup 1.0000x reference)
"""Trainium2 Bass kernel for capsule dynamic routing (nn_Capsule).

Math (per sample):
  hat[i,(n,d)] = sum_d' x[i,d'] W[d',(n,d)]        (i=1024, d'=128, n=32, d=16)
  3 routing iters: c = softmax(b, axis=n); o = squash(sum_i c[n,i] hat[i,n,:])
                   b = sum_d o[n,d] hat[i,n,d]
Never materialize hat.  W columns permuted k' = d*32 + n so masked reduces are
contiguous and the mask is one [128,128] tile for every chunk.

Per group of 4 samples (stacked 4*32 = 128 partitions q=(b,n)) and iteration,
stages (emitted software-pipelined with a 1-stage skew between groups):
  S0: GT[d',q] += xn-chunk^T-stationary MMs;  GTs drain.  At it=0 c is
      uniform so G has only 4 distinct columns: 32 N=1 MMs + bcast drain.
  S1: FT chunks (PE);  tsTu = FT*maskT (DVE);  sq = tsTu^2 (ACT);
      ss[q,1] = sq-stationary x ones-col MMs (PE, 4 moving rows)
  S2: squash scale sqrt(s)/(0.5+s): bit-trick rsqrt+recip Newton chain on
      Pool, PAIRED across two groups ([128,2] ops, half the invocations);
      scale broadcast along partitions via ACT bcast-mul + PE transpose
  S3: HT += wtp^T MMs (PE);  HTs = HTu*scBs fused scaled drain (DVE)
  S4: bt = xT-chunk MMs (PE);  exp (ACT, contiguous)
  S5: z = bf16 pair-add (2x) + reduce (DVE);  rz recip;  ct = e*rz
      (DVE/Pool halves)
Final iter: S0; F (512-col MM); masked reduce -> o;  o = s*scale -> DMA out.
Sharding: data-parallel over batch, 16 samples/core x 8 cores.
"""

import os
import sys

sys.path.insert(0, "/opt/trn_rl_repo")

import numpy as np

import concourse.bass as bass
import concourse.bacc as bacc
import concourse.mybir as mybir
from concourse import tile
from concourse.bass_utils import run_bass_kernel_spmd

FP32 = mybir.dt.float32
BF16 = mybir.dt.bfloat16
I32 = mybir.dt.int32
AF = mybir.ActivationFunctionType
AX = mybir.AxisListType
AL = mybir.AluOpType

EPS = 1e-7
N_CORES = 8
B_TOTAL, IN, D = 128, 1024, 128
NCAP, DC = 32, 16
K = NCAP * DC
B_LOC = B_TOTAL // N_CORES
GSZ = 4
NG = B_LOC // GSZ
NCH = IN // 128


def build():
    nc = bacc.Bacc("TRN2", target_bir_lowering=False)
    xT = nc.declare_dram_parameter("xT", [NG, D, GSZ, IN], BF16, isOutput=False)
    xn = nc.declare_dram_parameter("xn", [NG, 128, GSZ, NCH, D], BF16, isOutput=False)
    wp = nc.declare_dram_parameter("wp", [D, K], BF16, isOutput=False)
    wpc = nc.declare_dram_parameter("wpc", [D, 4, 128], BF16, isOutput=False)
    wtp = nc.declare_dram_parameter("wtp", [K, D], BF16, isOutput=False)
    maskp = nc.declare_dram_parameter("maskp", [128, K], BF16, isOutput=False)
    maskt = nc.declare_dram_parameter("maskt", [128, 128], BF16, isOutput=False)
    ident = nc.declare_dram_parameter("ident", [128, 128], BF16, isOutput=False)
    out = nc.declare_dram_parameter("out", [B_LOC, NCAP, DC], FP32, isOutput=True)

    with tile.TileContext(nc) as tc:
        with (
            tc.tile_pool(name="const", bufs=1) as cpool,
            tc.tile_pool(name="xp", bufs=1) as xp,
            tc.tile_pool(name="sbp", bufs=6) as sbp,
            tc.tile_pool(name="tsp", bufs=8) as tsp,
            tc.tile_pool(name="ep", bufs=6) as ep,
            tc.tile_pool(name="ctp", bufs=8) as ctp,
            tc.tile_pool(name="small", bufs=16) as smallp,
            tc.tile_pool(name="gt", bufs=2, space="PSUM") as gtp,
            tc.tile_pool(name="ft", bufs=1, space="PSUM") as ftp,
            tc.tile_pool(name="sc", bufs=1, space="PSUM") as scp,
            tc.tile_pool(name="ht", bufs=1, space="PSUM") as htp,
            tc.tile_pool(name="bt", bufs=1, space="PSUM") as btp,
        ):
            # xn group 0 first so the pipeline ramps immediately
            xn_t = []
            for g in range(NG):
                t2 = xp.tile([128, GSZ, NCH, D], BF16, tag=f"xn{g}",
                             name=f"xng{g}")
                xn_t.append(t2)
            nc.sync.dma_start(xn_t[0][:], xn[0])
            wp_sb = cpool.tile([D, K], BF16, tag="wp")
            nc.sync.dma_start(wp_sb[:], wp[:])
            wpc_sb = cpool.tile([D, 4, 128], BF16, tag="wpc")
            nc.sync.dma_start(wpc_sb[:], wpc[:])
            wtp_sb = cpool.tile([128, 4, D], BF16, tag="wtp")
            nc.sync.dma_start(wtp_sb[:], wtp.rearrange("(j p) d -> p j d", p=128))
            mt_sb = cpool.tile([128, 128], BF16, tag="maskt")
            nc.sync.dma_start(mt_sb[:], maskt[:])
            id_sb = cpool.tile([128, 128], BF16, tag="ident")
            nc.sync.dma_start(id_sb[:], ident[:])
            c0_sb = cpool.tile([128, NCAP], BF16, tag="c0")
            nc.vector.memset(c0_sb[:], 1.0 / NCAP)
            ones_col = cpool.tile([128, 1], BF16, tag="ones_col")
            nc.vector.memset(ones_col[:], 1.0)
            ones128 = cpool.tile([128, 128], BF16, tag="ones128")
            nc.vector.memset(ones128[:], 1.0)
            # [128,1] constants for the Pool-engine squash tail (Pool has no
            # tensor_scalar, so everything is tensor_tensor against these)
            c05 = cpool.tile([128, 1], FP32, tag="c05")
            nc.vector.memset(c05[:], 0.5)
            c15 = cpool.tile([128, 1], FP32, tag="c15")
            nc.vector.memset(c15[:], 1.5)
            c2 = cpool.tile([128, 1], FP32, tag="c2")
            nc.vector.memset(c2[:], 2.0)
            mgR = cpool.tile([128, 1], I32, tag="mgR")
            nc.vector.memset(mgR[:], 0x5F3759DF)
            mgI = cpool.tile([128, 1], I32, tag="mgI")
            nc.vector.memset(mgI[:], 0x7EF311C3)

            xn_g = xn_t
            xT_g = []
            for g in range(NG):
                t = xp.tile([128, GSZ, IN], BF16, tag=f"xT{g}", name=f"xTg{g}")
                xT_g.append(t)
            # interleave remaining xn with xT in pipeline-need order: group
            # g needs xn_g at stage-round g (s0) and xT_g at round 4+g (s4)
            order = [("xn", 1), ("xT", 0), ("xn", 2), ("xT", 1),
                     ("xn", 3), ("xT", 2), ("xT", 3)]
            for kind, g in order:
                if kind == "xn":
                    nc.sync.dma_start(xn_g[g][:], xn[g])
                else:
                    nc.sync.dma_start(xT_g[g][:], xT[g])
            # maskp is only needed from the final iteration onwards
            mp_sb = cpool.tile([128, K], BF16, tag="maskp")
            nc.sync.dma_start(mp_sb[:], maskp[:])

            # persistent cross-stage state, per group / per group-pair
            st = [dict() for _ in range(NG)]
            pst = [dict() for _ in range(NG // 2)]
            ct = [None] * NG

            def squash_tail(ss_s, w):
                """sc = sqrt(s)/(0.5+s) from s=[128,w] fp32 SBUF.

                Two INDEPENDENT branches run concurrently: the Quake
                rsqrt+Newton chain on DVE (consecutive same-engine ops need
                no semaphores) and the bit-trick reciprocal of (0.5+s) on
                Pool.  They join in the final multiply on DVE.
                """
                p = smallp
                VE, GE = nc.vector, nc.gpsimd
                # --- rsqrt branch (DVE): sqrt(s) = s * rsqrt(s)
                ib = p.tile([128, w], I32, tag="ib")
                VE.tensor_scalar(ib[:], ss_s.bitcast(I32), 1, None,
                                 op0=AL.arith_shift_right)
                VE.tensor_scalar(ib[:], ib[:], -1, 0x5F3759DF,
                                 op0=AL.mult, op1=AL.add)
                y0 = ib[:].bitcast(FP32)
                h = p.tile([128, w], FP32, tag="h")
                VE.tensor_mul(h[:], y0, y0)
                VE.tensor_mul(h[:], h[:], ss_s)
                VE.tensor_scalar(h[:], h[:], -0.5, 1.5,
                                 op0=AL.mult, op1=AL.add)
                yN = p.tile([128, w], FP32, tag="yN")
                VE.tensor_mul(yN[:], y0, h[:])
                sv = p.tile([128, w], FP32, tag="sv")
                VE.tensor_mul(sv[:], yN[:], ss_s)
                # --- reciprocal branch (Pool): 1/(0.5+s)
                cw = lambda t: t[:, 0:1].to_broadcast([128, w]) if w > 1 else t[:]
                den = p.tile([128, w], FP32, tag="den")
                GE.tensor_add(den[:], ss_s, cw(c05))
                ri = p.tile([128, w], I32, tag="ri")
                GE.tensor_sub(ri[:], cw(mgI), den[:].bitcast(I32))
                r0 = ri[:].bitcast(FP32)
                e1 = p.tile([128, w], FP32, tag="e1")
                GE.tensor_mul(e1[:], den[:], r0)
                GE.tensor_sub(e1[:], cw(c2), e1[:])
                rden = p.tile([128, w], FP32, tag="rden")
                GE.tensor_mul(rden[:], r0, e1[:])
                # --- join (DVE)
                sc = p.tile([128, w], FP32, tag="sc")
                VE.tensor_mul(sc[:], sv[:], rden[:])
                return sc

            def make_scB(g, sc1):
                # sc[q] broadcast along partitions: DVE bcast-mul (chains
                # sem-free after the tail's final DVE op) then PE transpose
                scBT = smallp.tile([128, 128], BF16, tag="scbt")
                nc.vector.tensor_scalar_mul(scBT[:], ones128[:], sc1)
                scB = scp.tile([128, 256], BF16, tag="scb", name="scBps")
                nc.tensor.transpose(scB[:, 0:128], scBT[:], id_sb[:])
                scBs = sbp.tile([128, 128], BF16, tag="scbs")
                nc.scalar.copy(scBs[:], scB[:, 0:128])
                st[g]["scBs"] = scBs

            def s0(g, it):
                GT4 = gtp.tile([128, 128], FP32, tag="gt4")
                Gs = sbp.tile([128, 128], BF16, tag="gts")
                if it == 0:
                    # c uniform: G has 4 distinct columns (one per sample)
                    for b in range(GSZ):
                        for c in range(NCH):
                            nc.tensor.matmul(
                                GT4[:, b:b + 1],
                                xn_g[g][:, b, c, :],
                                c0_sb[:, 0:1],
                                start=(c == 0),
                                stop=(c == NCH - 1),
                            )
                    nc.scalar.copy(
                        Gs[:].rearrange("p (b n) -> p b n", b=GSZ),
                        GT4[:, 0:GSZ].rearrange("p (b x) -> p b x", x=1)
                        .to_broadcast([128, GSZ, NCAP]),
                    )
                else:
                    for b in range(GSZ):
                        for c in range(NCH):
                            nc.tensor.matmul(
                                GT4[:, 32 * b:32 * b + 32],
                                xn_g[g][:, b, c, :],
                                ct[g][:, b, c, :],
                                start=(c == 0),
                                stop=(c == NCH - 1),
                            )
                    nc.scalar.copy(Gs[:], GT4[:])
                st[g]["GTs"] = Gs

            def s1(g, it):
                # FT chunks; masked drain; squared; ss[q,1] via PE
                Gs = st[g].pop("GTs")
                FT4 = ftp.tile([128, 4, 128], FP32, tag="ft4t", name="FT4")
                for j in range(4):
                    nc.tensor.matmul(
                        FT4[:, j, :], wpc_sb[:, j, :], Gs[:],
                        start=True, stop=True,
                    )
                tsTu = tsp.tile([128, 4, 128], BF16, tag="tstu")
                nc.vector.tensor_mul(
                    tsTu[:], FT4[:],
                    mt_sb[:].rearrange("p (a q) -> p a q", a=1)
                    .to_broadcast([128, 4, 128]),
                )
                sqT = tsp.tile([128, 4, 128], BF16, tag="sqt")
                nc.scalar.square(sqT[:], tsTu[:])
                # squash denom s[q] = sum_k sq[k,q]: sq-stationary x ones
                # moving -> [q,1] directly (4 moving rows total)
                ss4P = scp.tile([128, 1], FP32, tag="ss4p", name="ss4P")
                for j in range(4):
                    nc.tensor.matmul(
                        ss4P[:], sqT[:, j, :], ones_col[:],
                        start=(j == 0), stop=(j == 3),
                    )
                st[g]["tsTu"] = tsTu
                st[g]["ss4P"] = ss4P

            def s2(g, it):
                # drain ss (+EPS); per-group squash tail (UNPAIRED)
                ss4P = st[g].pop("ss4P")
                ss2 = smallp.tile([128, 1], FP32, tag="ss2")
                nc.vector.tensor_scalar_add(ss2[:], ss4P[:], EPS)
                sc2 = squash_tail(ss2[:], 1)
                make_scB(g, sc2[:, 0:1])

            def s3(g, it):
                tsTu = st[g].pop("tsTu")
                scBs = st[g].pop("scBs")
                HTu = htp.tile([128, 128], FP32, tag="htu")
                for j in range(4):
                    nc.tensor.matmul(
                        HTu[:], wtp_sb[:, j, :], tsTu[:, j, :],
                        start=(j == 0), stop=(j == 3),
                    )
                HTs = sbp.tile([128, 128], BF16, tag="hts")
                nc.vector.tensor_mul(HTs[:], HTu[:], scBs[:])
                st[g]["HTs"] = HTs

            def s4stage(g, it):
                HTs = st[g].pop("HTs")
                bt4 = btp.tile([128, GSZ, NCH, NCAP], FP32, tag="bt4")
                for b in range(GSZ):
                    for c in range(NCH):
                        nc.tensor.matmul(
                            bt4[:, b, c, :],
                            xT_g[g][:, b, 128 * c:128 * c + 128],
                            HTs[:, 32 * b:32 * b + 32],
                            start=True,
                            stop=True,
                        )
                e4 = ep.tile([128, GSZ, NCH, NCAP], BF16, tag="e4")
                nc.scalar.activation(e4[:], bt4[:], AF.Exp)
                st[g]["e4"] = e4

            def s5(g, it):
                e4 = st[g].pop("e4")
                z4f = smallp.tile([128, GSZ, NCH], FP32, tag="z4f")
                nc.vector.tensor_reduce(z4f[:], e4[:], axis=AX.X, op=AL.add)
                rz4 = smallp.tile([128, GSZ, NCH], BF16, tag="rz4")
                with nc.allow_low_precision("softmax denominators O(1-30)"):
                    nc.vector.reciprocal(rz4[:], z4f[:])
                ctg = ctp.tile([128, GSZ, NCH, NCAP], BF16, tag="ct4")
                rzv = rz4[:]
                nc.vector.tensor_mul(
                    ctg[:], e4[:],
                    rzv[:].to_broadcast([128, GSZ, NCH, NCAP]),
                )
                ct[g] = ctg

            def s1b(g, it):
                # final iter: full F, masked segment-reduce -> o (unsquashed)
                Gs = st[g].pop("GTs")
                F4t = ftp.tile([128, 4, 128], FP32, tag="ft4t", name="FT4")
                F4 = F4t[:].rearrange("p j q -> p (j q)")
                nc.tensor.matmul(F4, Gs[:], wp_sb[:], start=True, stop=True)
                ts4 = tsp.tile([128, K], BF16, tag="ts4")
                nc.vector.tensor_mul(ts4[:], F4, mp_sb[:])
                s4 = smallp.tile([128, DC], FP32, tag="s4")
                nc.vector.tensor_reduce(
                    s4[:], ts4[:].rearrange("p (d n) -> p d n", d=DC),
                    axis=AX.X, op=AL.add,
                )
                ssb = smallp.tile([128, 1], FP32, tag="ssb")
                dump = smallp.tile([128, DC], FP32, tag="sqdump")
                nc.scalar.activation(dump[:], s4[:], AF.Square,
                                     accum_out=ssb[:])
                st[g]["ssb1"] = ssb
                st[g]["s4"] = s4

            def s2b(g, it):
                sc2 = squash_tail(st[g].pop("ssb1")[:], 1)
                s4 = st[g].pop("s4")
                o4 = smallp.tile([128, DC], FP32, tag="o4")
                nc.vector.tensor_scalar_mul(o4[:], s4[:], sc2[:, 0:1])
                nc.sync.dma_start(
                    out[g * GSZ:(g + 1) * GSZ].rearrange("b n d -> (b n) d"),
                    o4[:],
                )

            # stage list per group: 3 iterations, last one truncated
            STAGES = []
            for it in range(2):
                STAGES += [(s0, it), (s1, it), (s2, it), (s3, it),
                           (s4stage, it), (s5, it)]
            STAGES += [(s0, 2), (s1b, 2), (s2b, 2)]

            NS = len(STAGES)
            SKEW = 2
            # descending g: within a round the earlier-stage (higher-g)
            # group emits first, so the paired squash tail (emitted at the
            # odd group's s2) lands before the even group's s3 consumes it
            for r in range(NS + SKEW * (NG - 1)):
                for g in range(NG):
                    s = r - SKEW * g
                    if 0 <= s < NS:
                        fn, it = STAGES[s]
                        fn(g, it)
    nc.compile()
    return nc


LAST_RESULT = None
_CONSTS = None


def _consts():
    global _CONSTS
    if _CONSTS is None:
        perm = np.empty(K, np.int64)
        for n in range(NCAP):
            for d in range(DC):
                perm[d * NCAP + n] = n * DC + d
        m32 = np.tile(np.eye(NCAP, dtype=np.float32), (1, DC)).reshape(NCAP, K)
        maskp = np.tile(m32, (GSZ, 1))
        pp, qq = np.meshgrid(np.arange(128), np.arange(128), indexing="ij")
        maskt = (pp % 32 == qq % 32).astype(np.float32)
        _CONSTS = (perm, maskp, maskt)
    return _CONSTS


def kernel(inputs, kernel):
    import ml_dtypes
    bf16 = ml_dtypes.bfloat16
    x = np.ascontiguousarray(np.asarray(inputs, dtype=np.float32))
    W = np.ascontiguousarray(np.asarray(kernel, dtype=np.float32)[0])
    # p-major per group of GSZ samples so every group DMA is contiguous
    xTh = np.ascontiguousarray(
        x.reshape(B_TOTAL // GSZ, GSZ, IN, D).transpose(0, 3, 1, 2).astype(bf16)
    )  # [B/GSZ, D, GSZ, IN]
    xnL = np.ascontiguousarray(
        x.reshape(B_TOTAL // GSZ, GSZ, NCH, 128, D).transpose(0, 3, 1, 2, 4)
        .astype(bf16)
    )  # [B/GSZ, 128, GSZ, NCH, D]
    perm, maskp, maskt = _consts()
    WPf = W[:, perm]
    WP = np.ascontiguousarray(WPf.astype(bf16))
    WPC = np.ascontiguousarray(WPf.reshape(D, 4, 128).astype(bf16))
    WTP = np.ascontiguousarray(WPf.T.astype(bf16))

    nc = build()
    in_maps = [
        {
            "xT": xTh[i * NG:(i + 1) * NG],
            "xn": xnL[i * NG:(i + 1) * NG],
            "wp": WP,
            "wpc": WPC,
            "wtp": WTP,
            "maskp": maskp.astype(bf16),
            "maskt": maskt.astype(bf16),
            "ident": np.eye(128, dtype=np.float32).astype(bf16),
        }
        for i in range(N_CORES)
    ]
    res = run_bass_kernel_spmd(
        nc, in_maps, core_ids=list(range(N_CORES)),
        trace=bool(os.environ.get("KERNEL_TRACE")),
    )
    global LAST_RESULT
    LAST_RESULT = res
    return np.concatenate([res.results[i]["out"] for i in range(N_CORES)], axis=0)


if __name__ == "__main__":
    rng = np.random.default_rng(0)
    xi = rng.standard_normal((B_TOTAL, IN, D), dtype=np.float32)
    ki = (rng.standard_normal((1, D, K), dtype=np.float32) * 0.05).astype(np.float32)
    o = kernel(xi, ki)
    print(o.shape, o.dtype)


# revision 69
# speedup vs baseline: 1.0441x; 1.0441x over previous
"""Trainium2 Bass kernel for capsule dynamic routing (nn_Capsule).

Math (per sample):
  hat[i,(n,d)] = sum_d' x[i,d'] W[d',(n,d)]        (i=1024, d'=128, n=32, d=16)
  3 routing iters: c = softmax(b, axis=n); o = squash(sum_i c[n,i] hat[i,n,:])
                   b = sum_d o[n,d] hat[i,n,d]
Never materialize hat.  W columns permuted k' = d*32 + n so masked reduces are
contiguous and the mask is one [128,128] tile for every chunk.

Per group of 4 samples (stacked 4*32 = 128 partitions q=(b,n)) and iteration,
stages (emitted software-pipelined with a 1-stage skew between groups):
  S0: GT[d',q] += xn-chunk^T-stationary MMs;  GTs drain.  At it=0 c is
      uniform so G has only 4 distinct columns: 32 N=1 MMs + bcast drain.
  S1: FT chunks (PE);  tsTu = FT*maskT (DVE);  sq = tsTu^2 (ACT);
      ss[q,1] = sq-stationary x ones-col MMs (PE, 4 moving rows)
  S2: squash scale sqrt(s)/(0.5+s): bit-trick rsqrt+recip Newton chain on
      Pool, PAIRED across two groups ([128,2] ops, half the invocations);
      scale broadcast along partitions via ACT bcast-mul + PE transpose
  S3: HT += wtp^T MMs (PE);  HTs = HTu*scBs fused scaled drain (DVE)
  S4: bt = xT-chunk MMs (PE);  exp (ACT, contiguous)
  S5: z = bf16 pair-add (2x) + reduce (DVE);  rz recip;  ct = e*rz
      (DVE/Pool halves)
Final iter: S0; F (512-col MM); masked reduce -> o;  o = s*scale -> DMA out.
Sharding: data-parallel over batch, 16 samples/core x 8 cores.
"""

import os
import sys

sys.path.insert(0, "/opt/trn_rl_repo")

import numpy as np

import concourse.bass as bass
import concourse.bacc as bacc
import concourse.mybir as mybir
from concourse import tile
from concourse.bass_utils import run_bass_kernel_spmd

FP32 = mybir.dt.float32
BF16 = mybir.dt.bfloat16
I32 = mybir.dt.int32
AF = mybir.ActivationFunctionType
AX = mybir.AxisListType
AL = mybir.AluOpType

EPS = 1e-7
N_CORES = 8
B_TOTAL, IN, D = 128, 1024, 128
NCAP, DC = 32, 16
K = NCAP * DC
B_LOC = B_TOTAL // N_CORES
GSZ = 4
NG = B_LOC // GSZ
NCH = IN // 128


def build():
    nc = bacc.Bacc("TRN2", target_bir_lowering=False)
    xT = nc.declare_dram_parameter("xT", [NG, D, GSZ, IN], BF16, isOutput=False)
    xn = nc.declare_dram_parameter("xn", [NG, 128, GSZ, NCH, D], BF16, isOutput=False)
    wp = nc.declare_dram_parameter("wp", [D, K], BF16, isOutput=False)
    wpc = nc.declare_dram_parameter("wpc", [D, 4, 128], BF16, isOutput=False)
    wtp = nc.declare_dram_parameter("wtp", [K, D], BF16, isOutput=False)
    maskp = nc.declare_dram_parameter("maskp", [128, K], BF16, isOutput=False)
    maskt = nc.declare_dram_parameter("maskt", [128, 128], BF16, isOutput=False)
    ident = nc.declare_dram_parameter("ident", [128, 128], BF16, isOutput=False)
    out = nc.declare_dram_parameter("out", [B_LOC, NCAP, DC], FP32, isOutput=True)

    with tile.TileContext(nc) as tc:
        with (
            tc.tile_pool(name="const", bufs=1) as cpool,
            tc.tile_pool(name="xp", bufs=1) as xp,
            tc.tile_pool(name="sbp", bufs=6) as sbp,
            tc.tile_pool(name="tsp", bufs=8) as tsp,
            tc.tile_pool(name="ep", bufs=6) as ep,
            tc.tile_pool(name="ctp", bufs=8) as ctp,
            tc.tile_pool(name="small", bufs=16) as smallp,
            tc.tile_pool(name="gt", bufs=2, space="PSUM") as gtp,
            tc.tile_pool(name="ft", bufs=1, space="PSUM") as ftp,
            tc.tile_pool(name="sc", bufs=1, space="PSUM") as scp,
            tc.tile_pool(name="ht", bufs=1, space="PSUM") as htp,
            tc.tile_pool(name="bt", bufs=1, space="PSUM") as btp,
        ):
            # xn group 0 first so the pipeline ramps immediately
            xn_t = []
            for g in range(NG):
                t2 = xp.tile([128, GSZ, NCH, D], BF16, tag=f"xn{g}",
                             name=f"xng{g}")
                xn_t.append(t2)
            nc.sync.dma_start(xn_t[0][:], xn[0])
            wp_sb = cpool.tile([D, K], BF16, tag="wp")
            nc.sync.dma_start(wp_sb[:], wp[:])
            wpc_sb = cpool.tile([D, 4, 128], BF16, tag="wpc")
            nc.sync.dma_start(wpc_sb[:], wpc[:])
            wtp_sb = cpool.tile([128, 4, D], BF16, tag="wtp")
            nc.sync.dma_start(wtp_sb[:], wtp.rearrange("(j p) d -> p j d", p=128))
            mt_sb = cpool.tile([128, 128], BF16, tag="maskt")
            nc.sync.dma_start(mt_sb[:], maskt[:])
            id_sb = cpool.tile([128, 128], BF16, tag="ident")
            nc.sync.dma_start(id_sb[:], ident[:])
            c0_sb = cpool.tile([128, NCAP], BF16, tag="c0")
            nc.vector.memset(c0_sb[:], 1.0 / NCAP)
            ones_col = cpool.tile([128, 1], BF16, tag="ones_col")
            nc.vector.memset(ones_col[:], 1.0)
            ones128 = cpool.tile([128, 128], BF16, tag="ones128")
            nc.vector.memset(ones128[:], 1.0)
            # [128,1] constants for the Pool-engine squash tail (Pool has no
            # tensor_scalar, so everything is tensor_tensor against these)
            c05 = cpool.tile([128, 1], FP32, tag="c05")
            nc.vector.memset(c05[:], 0.5)
            c15 = cpool.tile([128, 1], FP32, tag="c15")
            nc.vector.memset(c15[:], 1.5)
            c2 = cpool.tile([128, 1], FP32, tag="c2")
            nc.vector.memset(c2[:], 2.0)
            mgR = cpool.tile([128, 1], I32, tag="mgR")
            nc.vector.memset(mgR[:], 0x5F3759DF)
            mgI = cpool.tile([128, 1], I32, tag="mgI")
            nc.vector.memset(mgI[:], 0x7EF311C3)

            xn_g = xn_t
            xT_g = []
            for g in range(NG):
                t = xp.tile([128, GSZ, IN], BF16, tag=f"xT{g}", name=f"xTg{g}")
                xT_g.append(t)
            # interleave remaining xn with xT in pipeline-need order: group
            # g needs xn_g at stage-round g (s0) and xT_g at round 4+g (s4)
            order = [("xn", 1), ("xT", 0), ("xn", 2), ("xT", 1),
                     ("xn", 3), ("xT", 2), ("xT", 3)]
            for kind, g in order:
                if kind == "xn":
                    nc.sync.dma_start(xn_g[g][:], xn[g])
                else:
                    nc.sync.dma_start(xT_g[g][:], xT[g])
            # maskp is only needed from the final iteration onwards
            mp_sb = cpool.tile([128, K], BF16, tag="maskp")
            nc.sync.dma_start(mp_sb[:], maskp[:])

            # persistent cross-stage state, per group / per group-pair
            st = [dict() for _ in range(NG)]
            pst = [dict() for _ in range(NG // 2)]
            ct = [None] * NG

            def squash_tail(ss_s, w):
                """sc = sqrt(s)/(0.5+s) from s=[128,w] fp32 SBUF.

                Two INDEPENDENT branches run concurrently: the Quake
                rsqrt+Newton chain on DVE (consecutive same-engine ops need
                no semaphores) and the bit-trick reciprocal of (0.5+s) on
                Pool.  They join in the final multiply on DVE.
                """
                p = smallp
                VE, GE = nc.vector, nc.gpsimd
                # --- rsqrt branch (DVE): sqrt(s) = s * rsqrt(s)
                ib = p.tile([128, w], I32, tag="ib")
                VE.tensor_scalar(ib[:], ss_s.bitcast(I32), 1, None,
                                 op0=AL.arith_shift_right)
                VE.tensor_scalar(ib[:], ib[:], -1, 0x5F3759DF,
                                 op0=AL.mult, op1=AL.add)
                y0 = ib[:].bitcast(FP32)
                h = p.tile([128, w], FP32, tag="h")
                VE.tensor_mul(h[:], y0, y0)
                VE.tensor_mul(h[:], h[:], ss_s)
                VE.tensor_scalar(h[:], h[:], -0.5, 1.5,
                                 op0=AL.mult, op1=AL.add)
                yN = p.tile([128, w], FP32, tag="yN")
                VE.tensor_mul(yN[:], y0, h[:])
                sv = p.tile([128, w], FP32, tag="sv")
                VE.tensor_mul(sv[:], yN[:], ss_s)
                # --- reciprocal branch (Pool): 1/(0.5+s)
                cw = lambda t: t[:, 0:1].to_broadcast([128, w]) if w > 1 else t[:]
                den = p.tile([128, w], FP32, tag="den")
                GE.tensor_add(den[:], ss_s, cw(c05))
                ri = p.tile([128, w], I32, tag="ri")
                GE.tensor_sub(ri[:], cw(mgI), den[:].bitcast(I32))
                r0 = ri[:].bitcast(FP32)
                e1 = p.tile([128, w], FP32, tag="e1")
                GE.tensor_mul(e1[:], den[:], r0)
                GE.tensor_sub(e1[:], cw(c2), e1[:])
                rden = p.tile([128, w], FP32, tag="rden")
                GE.tensor_mul(rden[:], r0, e1[:])
                # --- join (DVE)
                sc = p.tile([128, w], FP32, tag="sc")
                VE.tensor_mul(sc[:], sv[:], rden[:])
                return sc

            def make_scB(g, sc1):
                # sc[q] broadcast along partitions: DVE bcast-mul (chains
                # sem-free after the tail's final DVE op) then PE transpose
                scBT = smallp.tile([128, 128], BF16, tag="scbt")
                nc.vector.tensor_scalar_mul(scBT[:], ones128[:], sc1)
                scB = scp.tile([128, 256], BF16, tag="scb", name="scBps")
                nc.tensor.transpose(scB[:, 0:128], scBT[:], id_sb[:])
                scBs = sbp.tile([128, 128], BF16, tag="scbs")
                nc.scalar.copy(scBs[:], scB[:, 0:128])
                st[g]["scBs"] = scBs

            def s0(g, it):
                GT4 = gtp.tile([128, 128], FP32, tag="gt4")
                Gs = sbp.tile([128, 128], BF16, tag="gts")
                if it == 0:
                    # c uniform: G has 4 distinct columns (one per sample)
                    for b in range(GSZ):
                        for c in range(NCH):
                            nc.tensor.matmul(
                                GT4[:, b:b + 1],
                                xn_g[g][:, b, c, :],
                                c0_sb[:, 0:1],
                                start=(c == 0),
                                stop=(c == NCH - 1),
                            )
                    nc.scalar.copy(
                        Gs[:].rearrange("p (b n) -> p b n", b=GSZ),
                        GT4[:, 0:GSZ].rearrange("p (b x) -> p b x", x=1)
                        .to_broadcast([128, GSZ, NCAP]),
                    )
                else:
                    for b in range(GSZ):
                        for c in range(NCH):
                            nc.tensor.matmul(
                                GT4[:, 32 * b:32 * b + 32],
                                xn_g[g][:, b, c, :],
                                ct[g][:, b, c, :],
                                start=(c == 0),
                                stop=(c == NCH - 1),
                            )
                    nc.scalar.copy(Gs[:], GT4[:])
                st[g]["GTs"] = Gs

            def s1(g, it):
                # FT chunks; masked drain; squared; ss[q,1] via PE
                Gs = st[g].pop("GTs")
                FT4 = ftp.tile([128, 4, 128], FP32, tag="ft4t", name="FT4")
                for j in range(4):
                    nc.tensor.matmul(
                        FT4[:, j, :], wpc_sb[:, j, :], Gs[:],
                        start=True, stop=True,
                    )
                tsTu = tsp.tile([128, 4, 128], BF16, tag="tstu")
                nc.vector.tensor_mul(
                    tsTu[:], FT4[:],
                    mt_sb[:].rearrange("p (a q) -> p a q", a=1)
                    .to_broadcast([128, 4, 128]),
                )
                sqT = tsp.tile([128, 4, 128], BF16, tag="sqt")
                nc.scalar.square(sqT[:], tsTu[:])
                # squash denom s[q] = sum_k sq[k,q]: sq-stationary x ones
                # moving -> [q,1] directly (4 moving rows total)
                ss4P = scp.tile([128, 1], FP32, tag="ss4p", name="ss4P")
                for j in range(4):
                    nc.tensor.matmul(
                        ss4P[:], sqT[:, j, :], ones_col[:],
                        start=(j == 0), stop=(j == 3),
                    )
                st[g]["tsTu"] = tsTu
                st[g]["ss4P"] = ss4P

            def s2(g, it):
                # drain ss (+EPS); per-group squash tail (UNPAIRED)
                ss4P = st[g].pop("ss4P")
                ss2 = smallp.tile([128, 1], FP32, tag="ss2")
                nc.vector.tensor_scalar_add(ss2[:], ss4P[:], EPS)
                sc2 = squash_tail(ss2[:], 1)
                make_scB(g, sc2[:, 0:1])

            def s3(g, it):
                tsTu = st[g].pop("tsTu")
                scBs = st[g].pop("scBs")
                HTu = htp.tile([128, 128], FP32, tag="htu")
                for j in range(4):
                    nc.tensor.matmul(
                        HTu[:], wtp_sb[:, j, :], tsTu[:, j, :],
                        start=(j == 0), stop=(j == 3),
                    )
                HTs = sbp.tile([128, 128], BF16, tag="hts")
                nc.vector.tensor_mul(HTs[:], HTu[:], scBs[:])
                st[g]["HTs"] = HTs

            def s4stage(g, it):
                HTs = st[g].pop("HTs")
                bt4 = btp.tile([128, GSZ, NCH, NCAP], FP32, tag="bt4")
                for b in range(GSZ):
                    for c in range(NCH):
                        nc.tensor.matmul(
                            bt4[:, b, c, :],
                            xT_g[g][:, b, 128 * c:128 * c + 128],
                            HTs[:, 32 * b:32 * b + 32],
                            start=True,
                            stop=True,
                        )
                e4 = ep.tile([128, GSZ, NCH, NCAP], BF16, tag="e4")
                nc.scalar.activation(e4[:], bt4[:], AF.Exp)
                st[g]["e4"] = e4

            def s5(g, it):
                e4 = st[g].pop("e4")
                # first tree level in bf16 2x mode, then contiguous reduce
                t16 = tsp.tile([128, GSZ, NCH, 16], BF16, tag="t16")
                nc.vector.tensor_add(t16[:], e4[:, :, :, 0:16],
                                     e4[:, :, :, 16:32])
                t8 = tsp.tile([128, GSZ, NCH, 8], BF16, tag="t8")
                nc.vector.tensor_add(t8[:], t16[:, :, :, 0:8],
                                     t16[:, :, :, 8:16])
                z4f = smallp.tile([128, GSZ, NCH], FP32, tag="z4f")
                nc.vector.tensor_reduce(z4f[:], t8[:], axis=AX.X, op=AL.add)
                rz4 = smallp.tile([128, GSZ, NCH], BF16, tag="rz4")
                with nc.allow_low_precision("softmax denominators O(1-30)"):
                    nc.vector.reciprocal(rz4[:], z4f[:])
                ctg = ctp.tile([128, GSZ, NCH, NCAP], BF16, tag="ct4")
                rzv = rz4[:]
                nc.vector.tensor_mul(
                    ctg[:], e4[:],
                    rzv[:].to_broadcast([128, GSZ, NCH, NCAP]),
                )
                ct[g] = ctg

            def s1b(g, it):
                # final iter: full F, masked segment-reduce -> o (unsquashed)
                Gs = st[g].pop("GTs")
                F4t = ftp.tile([128, 4, 128], FP32, tag="ft4t", name="FT4")
                F4 = F4t[:].rearrange("p j q -> p (j q)")
                nc.tensor.matmul(F4, Gs[:], wp_sb[:], start=True, stop=True)
                ts4 = tsp.tile([128, K], BF16, tag="ts4")
                nc.vector.tensor_mul(ts4[:], F4, mp_sb[:])
                s4 = smallp.tile([128, DC], FP32, tag="s4")
                nc.vector.tensor_reduce(
                    s4[:], ts4[:].rearrange("p (d n) -> p d n", d=DC),
                    axis=AX.X, op=AL.add,
                )
                ssb = smallp.tile([128, 1], FP32, tag="ssb")
                dump = smallp.tile([128, DC], FP32, tag="sqdump")
                nc.scalar.activation(dump[:], s4[:], AF.Square,
                                     accum_out=ssb[:])
                st[g]["ssb1"] = ssb
                st[g]["s4"] = s4

            def s2b(g, it):
                sc2 = squash_tail(st[g].pop("ssb1")[:], 1)
                s4 = st[g].pop("s4")
                o4 = smallp.tile([128, DC], FP32, tag="o4")
                nc.vector.tensor_scalar_mul(o4[:], s4[:], sc2[:, 0:1])
                nc.sync.dma_start(
                    out[g * GSZ:(g + 1) * GSZ].rearrange("b n d -> (b n) d"),
                    o4[:],
                )

            # stage list per group: 3 iterations, last one truncated
            STAGES = []
            for it in range(2):
                STAGES += [(s0, it), (s1, it), (s2, it), (s3, it),
                           (s4stage, it), (s5, it)]
            STAGES += [(s0, 2), (s1b, 2), (s2b, 2)]

            NS = len(STAGES)
            SKEW = 2
            # descending g: within a round the earlier-stage (higher-g)
            # group emits first, so the paired squash tail (emitted at the
            # odd group's s2) lands before the even group's s3 consumes it
            for r in range(NS + SKEW * (NG - 1)):
                for g in range(NG):
                    s = r - SKEW * g
                    if 0 <= s < NS:
                        fn, it = STAGES[s]
                        fn(g, it)
    nc.compile()
    return nc


LAST_RESULT = None
_CONSTS = None


def _consts():
    global _CONSTS
    if _CONSTS is None:
        perm = np.empty(K, np.int64)
        for n in range(NCAP):
            for d in range(DC):
                perm[d * NCAP + n] = n * DC + d
        m32 = np.tile(np.eye(NCAP, dtype=np.float32), (1, DC)).reshape(NCAP, K)
        maskp = np.tile(m32, (GSZ, 1))
        pp, qq = np.meshgrid(np.arange(128), np.arange(128), indexing="ij")
        maskt = (pp % 32 == qq % 32).astype(np.float32)
        _CONSTS = (perm, maskp, maskt)
    return _CONSTS


def kernel(inputs, kernel):
    import ml_dtypes
    bf16 = ml_dtypes.bfloat16
    x = np.ascontiguousarray(np.asarray(inputs, dtype=np.float32))
    W = np.ascontiguousarray(np.asarray(kernel, dtype=np.float32)[0])
    # p-major per group of GSZ samples so every group DMA is contiguous
    xTh = np.ascontiguousarray(
        x.reshape(B_TOTAL // GSZ, GSZ, IN, D).transpose(0, 3, 1, 2).astype(bf16)
    )  # [B/GSZ, D, GSZ, IN]
    xnL = np.ascontiguousarray(
        x.reshape(B_TOTAL // GSZ, GSZ, NCH, 128, D).transpose(0, 3, 1, 2, 4)
        .astype(bf16)
    )  # [B/GSZ, 128, GSZ, NCH, D]
    perm, maskp, maskt = _consts()
    WPf = W[:, perm]
    WP = np.ascontiguousarray(WPf.astype(bf16))
    WPC = np.ascontiguousarray(WPf.reshape(D, 4, 128).astype(bf16))
    WTP = np.ascontiguousarray(WPf.T.astype(bf16))

    nc = build()
    in_maps = [
        {
            "xT": xTh[i * NG:(i + 1) * NG],
            "xn": xnL[i * NG:(i + 1) * NG],
            "wp": WP,
            "wpc": WPC,
            "wtp": WTP,
            "maskp": maskp.astype(bf16),
            "maskt": maskt.astype(bf16),
            "ident": np.eye(128, dtype=np.float32).astype(bf16),
        }
        for i in range(N_CORES)
    ]
    res = run_bass_kernel_spmd(
        nc, in_maps, core_ids=list(range(N_CORES)),
        trace=bool(os.environ.get("KERNEL_TRACE")),
    )
    global LAST_RESULT
    LAST_RESULT = res
    return np.concatenate([res.results[i]["out"] for i in range(N_CORES)], axis=0)


if __name__ == "__main__":
    rng = np.random.default_rng(0)
    xi = rng.standard_normal((B_TOTAL, IN, D), dtype=np.float32)
    ki = (rng.standard_normal((1, D, K), dtype=np.float32) * 0.05).astype(np.float32)
    o = kernel(xi, ki)
    print(o.shape, o.dtype)
